# revision 23
# baseline (speedup 1.0000x reference)
"""Trainium2 Bass kernel for nn_Decode: masked-mean embed + fc/tanh + LSTM scan + output proj.

Strategy: pure data parallelism over batch (64 -> 8 cores x 8 sequences).
All heavy FLOPs on device; host only repacks weights and builds 0/1 index
matrices (selection matrix for the sliding-window mean, one-hot embedding
matrices, length mask) from the integer index inputs.

Key reformulations:
  - mean_emb@fc_w1^T is computed as (A @ (enc @ fc_w1^T)) where A[t,s] = 1/denom[t]
    for start[t] <= s < t, built host-side from wordlen_vals (banded 0/1 matrix).
  - embedding lookups are folded through fc: onehot @ (emb_w @ fc_w_part^T),
    with the fc bias as an extra ones-row.
  - LSTM input projections zp_t = z_t @ w_ih^T are precomputed for all t in fp8
    and injected into the per-step PSUM accumulation via a DoubleRow matmul
    whose stationary carries an identity + bias-row selector.
  - the recurrent h @ w_hh^T runs as fp8e4m3 DoubleRow matmuls (w_hh moving,
    h^T fp8 ring stationary): 256 effective contraction rows per instruction
    at 0.5 cycles/row.
  - the LSTM cell runs in bf16; tanh(c) is evaluated on the transposed c
    (PE transposes c and sigma(o) quadrants into PSUM) so h^T is produced
    directly in the fp8 ring layout the next step's matmul needs.
  - the final out = [h, enc] @ comb_w^T + comb_b runs in bf16 with the length
    mask folded into the PSUM->SBUF copy (scalar_tensor_tensor).
Gate order is permuted host-side to [i, f, o, g]; per-step issue order is
f, i, g, o so the f-gate (first consumer) completes first.
"""
import sys
import numpy as np

sys.path.insert(0, "/opt/trn_rl_repo")

B, T, E, H, LH = 64, 512, 1024, 512, 512
S = 66
PN, WN, PD, WD = 32, 8, 64, 64
NCORES = 8
BL = B // NCORES
G4 = 4 * H  # 2048
KEMB = PN + WN + 1  # 41 (pos onehot, wordlen onehot, bias row)
NZP = 2  # zp prefetch depth

_PROGRAM_CACHE = {}


def _build_program():
    from concourse import bass, tile, mybir
    from concourse import bacc

    f32 = mybir.dt.float32
    bf16 = mybir.dt.bfloat16
    f8 = mybir.dt.float8e4
    AF = mybir.ActivationFunctionType
    ALU = mybir.AluOpType
    DR = mybir.MatmulPerfMode.DoubleRow
    f32r = mybir.dt.float32r
    R = lambda ap: ap.bitcast(f32r)

    nc = bacc.Bacc("TRN2", target_bir_lowering=False)

    # ---------------- I/O ----------------
    encT_b = nc.declare_dram_parameter("encT_b", [BL, E, T], f32r, isOutput=False)
    encF = nc.declare_dram_parameter("encF", [E, T * BL], bf16, isOutput=False)
    ATp = nc.declare_dram_parameter("ATp", [BL, T, T], f32r, isOutput=False)
    onehotT = nc.declare_dram_parameter("onehotT", [BL, KEMB, T], f32r, isOutput=False)
    Rm = nc.declare_dram_parameter("Rm", [KEMB, LH], f32r, isOutput=False)
    fcw1T = nc.declare_dram_parameter("fcw1T", [E, LH], f32r, isOutput=False)
    wihT = nc.declare_dram_parameter("wihT", [LH, G4], f32r, isOutput=False)
    whh8p = nc.declare_dram_parameter("whh8p", [128, 2, 2, G4], f8, isOutput=False)
    l9f8p = nc.declare_dram_parameter("l9f8p", [8, 2, 16], f8, isOutput=False)
    zpbias = nc.declare_dram_parameter("zpbias", [8, G4], f8, isOutput=False)
    ident8 = nc.declare_dram_parameter("ident8", [BL, BL], bf16, isOutput=False)
    combT = nc.declare_dram_parameter("combT", [H + E, S], bf16, isOutput=False)
    combB = nc.declare_dram_parameter("combB", [S, 1], f32, isOutput=False)
    mask66 = nc.declare_dram_parameter("mask66", [S, T * BL], f32, isOutput=False)
    outP = nc.declare_dram_parameter("out", [S, T * BL], f32, isOutput=True)

    # ---------------- internal HBM ----------------
    zp_hbm = nc.dram_tensor("zp_hbm", [T, BL, G4], f8)

    with tile.TileContext(nc) as tc:
        # ============ persistent pools (live across whole kernel) ============
        with (
            tc.tile_pool(name="pers", bufs=1) as pers,
            tc.tile_pool(name="ring", bufs=1) as ringp,
            tc.tile_pool(name="cell", bufs=2) as cell,
        ):
            # scan weights resident: whh fp8 DoubleRow layout [p, i, j, n]
            whh8 = pers.tile([128, 2, 2, G4], f8, tag="whh8", name="whh8")
            nc.sync.dma_start(whh8[:], whh8p[:])
            l9_sb = pers.tile([8, 2, 16], f8, tag="l9", name="l9s")
            nc.sync.dma_start(l9_sb[:], l9f8p[:])
            id8_sb = pers.tile([BL, BL], bf16, tag="id8", name="id8_sb")
            nc.sync.dma_start(id8_sb[:], ident8[:])
            # zp staging tiles: [:, 0, :] = per-step zp, [:, 1, :] = bias row
            zp9 = [pers.tile([8, 2, G4], f8, tag=f"zp9_{i}", name=f"zp9_{i}") for i in range(NZP)]
            for i in range(NZP):
                nc.sync.dma_start(zp9[i][:, 1, :], zpbias[:])
            # h^T rings, fp8: [128, slot(64), q(4 = j*2+i), b(16 padded)]
            ring = [ringp.tile([128, 64, 4, 16], f8, tag=f"ring{p}", name=f"ring{p}") for p in range(2)]
            nc.vector.memset(ring[0][:].bitcast(f32), 0.0)
            nc.vector.memset(ring[1][:].bitcast(f32), 0.0)
            # LSTM cell state (bf16)
            c_sb = pers.tile([BL, H], bf16, tag="c_sb", name="c_sb")
            nc.vector.memset(c_sb[:].bitcast(f32), 0.0)

            # ================= pre-phases (per-sequence) =================
            with (
                tc.tile_pool(name="pre_w", bufs=1) as pre_w,
                tc.tile_pool(name="pre_s", bufs=1) as pre_s,
                tc.tile_pool(name="ps_pre", bufs=4, space="PSUM") as ps_pre,
            ):
                fcw1_sb = [pre_w.tile([128, LH], f32r, tag=f"fcw1_{e}", name=f"fcw1_{e}") for e in range(8)]
                for e in range(8):
                    nc.sync.dma_start(fcw1_sb[e][:], fcw1T[e * 128:(e + 1) * 128, :])
                wih_sb = [pre_w.tile([128, G4], f32r, tag=f"wih{k}", name=f"wih{k}") for k in range(4)]
                for k in range(4):
                    nc.sync.dma_start(wih_sb[k][:], wihT[k * 128:(k + 1) * 128, :])
                rm_sb = pre_w.tile([KEMB, LH], f32r, tag="rm", name="rm")
                nc.sync.dma_start(rm_sb[:], Rm[:])

                for b in range(BL):
                    # ---- load enc^T for this sequence ----
                    enc_sb = [pre_s.tile([128, T], f32r, tag=f"enc{e}", name=f"enc{e}") for e in range(8)]
                    for e in range(8):
                        nc.sync.dma_start(enc_sb[e][:], encT_b[b, e * 128:(e + 1) * 128, :])
                    # ---- P = enc @ fc_w1^T  -> [T(s), LH] ----
                    P_sb = [pre_s.tile([128, LH], f32r, tag=f"P{sc}", name=f"P{sc}") for sc in range(4)]
                    for sc in range(4):
                        ps = ps_pre.tile([128, LH], f32, tag="psA", name="psA")
                        for e in range(8):
                            nc.tensor.matmul(
                                ps[:], R(enc_sb[e][:, sc * 128:(sc + 1) * 128]),
                                R(fcw1_sb[e][:]), start=(e == 0), stop=(e == 7),
                            )
                        nc.vector.tensor_copy(P_sb[sc][:], ps[:])
                    # ---- z^T = tanh(P^T A^T + R^T onehot^T) -> [LH, T] ----
                    at_sb = [pre_s.tile([128, T], f32r, tag=f"at{sc}", name=f"at{sc}") for sc in range(4)]
                    for sc in range(4):
                        nc.sync.dma_start(at_sb[sc][:], ATp[b, sc * 128:(sc + 1) * 128, :])
                    oh_sb = pre_s.tile([KEMB, T], f32r, tag="oh", name="oh")
                    nc.sync.dma_start(oh_sb[:], onehotT[b, :, :])
                    zT_sb = [pre_s.tile([128, T], f32r, tag=f"zT{m}", name=f"zT{m}") for m in range(4)]
                    for m in range(4):
                        ps = ps_pre.tile([128, T], f32, tag="psA", name="psA2")
                        for sc in range(4):
                            nc.tensor.matmul(
                                ps[:], R(P_sb[sc][:, m * 128:(m + 1) * 128]),
                                R(at_sb[sc][:]), start=(sc == 0), stop=False,
                            )
                        nc.tensor.matmul(
                            ps[:], R(rm_sb[:, m * 128:(m + 1) * 128]), R(oh_sb[:]),
                            start=False, stop=True,
                        )
                        nc.scalar.activation(zT_sb[m][:], ps[:], AF.Tanh)
                        nc.vector.memset(zT_sb[m][:, 0:1].bitcast(f32), 0.0)  # z_0 = 0
                    # ---- zp = z @ w_ih^T -> HBM [T, b, 4H] fp8 ----
                    for mt in range(4):
                        for nb in range(4):
                            ps = ps_pre.tile([128, 512], f32, tag="psB", name="psB")
                            for kc in range(4):
                                nc.tensor.matmul(
                                    ps[:], R(zT_sb[kc][:, mt * 128:(mt + 1) * 128]),
                                    R(wih_sb[kc][:, nb * 512:(nb + 1) * 512]),
                                    start=(kc == 0), stop=(kc == 3),
                                )
                            zpo = pre_s.tile([128, 512], f8, tag="zpo", name="zpo", bufs=2)
                            nc.vector.tensor_copy(zpo[:], ps[:])
                            nc.sync.dma_start(
                                zp_hbm[mt * 128:(mt + 1) * 128, b, nb * 512:(nb + 1) * 512],
                                zpo[:],
                            )

            tc.strict_bb_all_engine_barrier()

            # ================= LSTM scan (+ fused output projection) =================
            scan_ctx = [
                tc.tile_pool(name="ps_scan", bufs=1, space="PSUM"),
                tc.tile_pool(name="ps_tp", bufs=2, space="PSUM"),
                tc.tile_pool(name="fin_w", bufs=1),
                tc.tile_pool(name="fin_s", bufs=3),
            ]
            ps_scan = scan_ctx[0].__enter__()
            ps_tp = scan_ctx[1].__enter__()
            fin_w = scan_ctx[2].__enter__()
            fin_s = scan_ctx[3].__enter__()
            cw_sb = [fin_w.tile([128, S], bf16, tag=f"cw{k}", name=f"cw{k}") for k in range(12)]
            for k in range(12):
                nc.sync.dma_start(cw_sb[k][:], combT[k * 128:(k + 1) * 128, :])
            cb_sb = fin_w.tile([S, 1], f32, tag="cb", name="cb")
            nc.sync.dma_start(cb_sb[:], combB[:])
            mk_sb = fin_w.tile([S, T * BL], f32, tag="mk", name="mk")
            nc.sync.dma_start(mk_sb[:], mask66[:])
            for i in range(NZP):
                nc.sync.dma_start(zp9[i][:, 0, :], zp_hbm[i])

            # gate banks (host perm [f, i, g, o]): bank nb = cols [nb*512, ...).
            # All sigmoids are tanh half-angle: sigma(x) = 0.5*tanh(x/2) + 0.5,
            # with the 0.5 input scales folded into w_ih/w_hh/bias host-side,
            # and h' = 2h (folded into w_hh rows and comb_w rows).
            BANK_ORDER = [0, 1, 2, 3]  # f, i, g, o

            def inject(t, nb):
                pg = ps_scan.tile([128, 512], f32, tag="gates", name="pgb", bufs=5)
                nc.tensor.matmul(
                    pg[0:16, :], l9_sb[:],
                    zp9[t % NZP][:, :, nb * 512:(nb + 1) * 512],
                    start=True, stop=False, perf_mode=DR,
                )
                return pg

            psb = {nb: inject(0, nb) for nb in BANK_ORDER}
            for t in range(T):
                par = (t // 64) % 2
                slot = t % 64
                pprev = ((t - 1) // 64) % 2
                sprev = (t - 1) % 64

                # recurrent fp8 DoubleRow matmuls for step t (f first)
                for nb in BANK_ORDER:
                    for j in range(2):
                        nc.tensor.matmul(
                            psb[nb][0:16, :],
                            ring[pprev][:, sprev, 2 * j:2 * j + 2, :],
                            whh8[:, :, j, nb * 512:(nb + 1) * 512],
                            start=False, stop=(j == 1), perf_mode=DR,
                        )

                # PE p-state fillers: keep the array streaming while the cell
                # chain runs (PE drops to low clock on idle gaps otherwise).
                dmy = ps_scan.tile([128, 512], f32, tag="gates", name="dmy", bufs=5)

                def filler(n):
                    for _ in range(n):
                        nc.tensor.matmul(
                            dmy[0:16, :],
                            ring[pprev][:, sprev, 0:2, :],
                            whh8[:, :, 0, 0:512],
                            start=True, stop=True, perf_mode=DR,
                        )

                filler(2)

                # tanh-only cell, bf16:
                #   cs' = 2c;  m2t = (tf+1)*cs';  m1 = (ti+1)*tg
                #   cs'_new = 0.5*m2t + m1;  h' = (to^T+1)*tanh(0.5*cs')^T
                tf = cell.tile([BL, H], bf16, tag="tf", name="tf")
                ti = cell.tile([BL, H], bf16, tag="ti", name="ti")
                tg = cell.tile([BL, H], bf16, tag="tg", name="tg")
                to = cell.tile([BL, H], bf16, tag="to", name="to")
                m1 = cell.tile([BL, H], bf16, tag="m1", name="m1")
                m2t = cell.tile([BL, H], bf16, tag="m2t", name="m2t")
                tcT = cell.tile([128, 4, 8], bf16, tag="tcT", name="tcT")
                nc.scalar.activation(tf[:], psb[0][0:8, :], AF.Tanh)
                nc.scalar.activation(ti[:], psb[1][0:8, :], AF.Tanh)
                nc.scalar.activation(tg[:], psb[2][0:8, :], AF.Tanh)
                nc.scalar.activation(to[:], psb[3][0:8, :], AF.Tanh)
                nc.vector.scalar_tensor_tensor(
                    m2t[:], tf[:], 1.0, c_sb[:], op0=ALU.add, op1=ALU.mult)
                nc.vector.scalar_tensor_tensor(
                    m1[:], ti[:], 1.0, tg[:], op0=ALU.add, op1=ALU.mult)
                nc.vector.scalar_tensor_tensor(
                    c_sb[:], m2t[:], 0.5, m1[:], op0=ALU.mult, op1=ALU.add)
                # transposed tail: sigma(o)^T first (only needs tanh_o — real
                # PE work in the gap), then c^T right when cs' lands; the
                # injects never sit in front of a chain-critical transpose.
                tp = ps_tp.tile([128, 2, 4, 8], bf16, tag="tp", name="tp")
                for q in range(4):
                    nc.tensor.transpose(
                        tp[:, 1, q, :],
                        to[:, q * 128:(q + 1) * 128], id8_sb[:],
                    )
                psb_next = {}
                if t + 1 < T:
                    psb_next[0] = inject(t + 1, 0)
                    psb_next[1] = inject(t + 1, 1)
                filler(1)
                for q in range(4):
                    nc.tensor.transpose(
                        tp[:, 0, q, :],
                        c_sb[:, q * 128:(q + 1) * 128], id8_sb[:],
                    )
                if t + 1 < T:
                    psb_next[2] = inject(t + 1, 2)
                    psb_next[3] = inject(t + 1, 3)
                nc.scalar.activation(tcT[:], tp[:, 0, :, :], AF.Tanh, scale=0.5)
                nc.vector.scalar_tensor_tensor(
                    ring[par][:, slot, :, 0:8],
                    tp[:, 1, :, :], 1.0, tcT[:], op0=ALU.add, op1=ALU.mult)

                if slot == 63:
                    ncol = t // 64
                    cols = slice(ncol * 512, (ncol + 1) * 512)
                    rb = fin_s.tile([128, 4, 64, 8], bf16, tag="rb", name="rb")
                    for q in range(4):
                        nc.vector.tensor_copy(
                            rb[:, q, :, :], ring[par][:, :, q, 0:8],
                        )
                    psf = ps_tp.tile([S, 512], f32, tag="psF", name="psF", bufs=1)
                    for q in range(4):
                        nc.tensor.matmul(
                            psf[:], cw_sb[q][:], rb[:, q, :, :],
                            start=(q == 0), stop=False,
                        )
                    for ke in range(8):
                        re = fin_s.tile([128, 512], bf16, tag="re", name="re")
                        nc.sync.dma_start(re[:], encF[ke * 128:(ke + 1) * 128, cols])
                        nc.tensor.matmul(
                            psf[:], cw_sb[4 + ke][:], re[:], start=False, stop=(ke == 7),
                        )
                    ob = fin_s.tile([S, 512], f32, tag="ob", name="ob")
                    nc.vector.scalar_tensor_tensor(
                        ob[:], psf[:], cb_sb[:, 0:1], mk_sb[:, cols],
                        op0=ALU.add, op1=ALU.mult,
                    )
                    if ncol == 0:
                        nc.vector.memset(ob[0:1, 0:8], -1e30)
                    nc.sync.dma_start(outP[:, cols], ob[:])
                if t + NZP < T:
                    nc.sync.dma_start(zp9[t % NZP][:, 0, :], zp_hbm[t + NZP])
                psb = psb_next

            for c in reversed(scan_ctx):
                c.__exit__(None, None, None)

    nc.compile()
    return nc


def _host_prepack(inputs):
    """Build per-core in_maps from full inputs. Index plumbing + weight repacking only."""
    import ml_dtypes
    f8t = ml_dtypes.float8_e4m3
    b16t = ml_dtypes.bfloat16
    enc = np.asarray(inputs["encode_out"], np.float32)
    pos_emb_w = np.asarray(inputs["pos_emb_w"], np.float32)
    wl_emb_w = np.asarray(inputs["wordlen_emb_w"], np.float32)
    fc_w = np.asarray(inputs["fc_w"], np.float32)
    fc_b = np.asarray(inputs["fc_b"], np.float32)
    w_ih = np.asarray(inputs["w_ih"], np.float32)
    w_hh = np.asarray(inputs["w_hh"], np.float32)
    b_ih = np.asarray(inputs["b_ih"], np.float32)
    b_hh = np.asarray(inputs["b_hh"], np.float32)
    comb_w = np.asarray(inputs["comb_w"], np.float32)
    comb_b = np.asarray(inputs["comb_b"], np.float32)
    pos_ids = np.asarray(inputs["pos_ids"])
    wl_ids = np.asarray(inputs["wordlen_ids"])
    wl_vals = np.asarray(inputs["wordlen_vals"])
    lengths = np.asarray(inputs["lengths"])

    t = np.arange(T)
    # gate permutation [f, i, g, o]; tanh half-angle scales: f,i,o inputs x0.5
    perm = np.r_[H:2 * H, 0:H, 2 * H:3 * H, 3 * H:4 * H]
    colscale = np.concatenate(
        [np.full(H, 0.5), np.full(H, 0.5), np.ones(H), np.full(H, 0.5)]
    ).astype(np.float32)
    wihT = np.ascontiguousarray(w_ih[perm].T) * colscale[None, :]  # [LH, 4H]
    # w_hh rows x0.5 compensates h' = 2h
    whhT = np.ascontiguousarray(w_hh[perm].T) * colscale[None, :] * 0.5
    # fp8 DoubleRow layout [p, i, j, n]: row K = j*256 + i*128 + p
    whh8 = np.ascontiguousarray(
        whhT.reshape(2, 2, 128, G4).transpose(2, 1, 0, 3)).astype(f8t)
    # inject stationary: [p, 0, m] = identity; [0, 1, m] = 1 (bias row)
    l9f8 = np.zeros((8, 2, 16), np.float32)
    l9f8[:, 0, :8] = np.eye(8)
    l9f8[0, 1, :8] = 1.0
    l9f8 = l9f8.astype(f8t)
    # bias row for zp9[:, 1, :]: partition 0 = b_ih + b_hh, rest 0
    zpbias = np.zeros((8, G4), np.float32)
    zpbias[0] = (b_ih + b_hh)[perm] * colscale
    zpbias = zpbias.astype(f8t)
    # embedding folding
    fc_w1 = fc_w[:, :E]
    fc_w2 = fc_w[:, E:E + WD]
    fc_w3 = fc_w[:, E + WD:]
    Rm = np.concatenate(
        [pos_emb_w @ fc_w3.T, wl_emb_w @ fc_w2.T, fc_b[None, :]], axis=0
    ).astype(np.float32)  # [41, LH]
    fcw1T = np.ascontiguousarray(fc_w1.T)  # [E, LH]
    combT = np.ascontiguousarray(comb_w.T)  # [1536, S]
    combT[:H] *= 0.5  # h' = 2h compensation
    combT = combT.astype(b16t)
    combB = comb_b[:, None].astype(np.float32)

    in_maps = []
    for c in range(NCORES):
        bs = slice(c * BL, (c + 1) * BL)
        enc_sh = enc[bs]  # [BL, T, E]
        encT_b = np.ascontiguousarray(enc_sh.transpose(0, 2, 1))  # [BL, E, T]
        encF = np.ascontiguousarray(
            enc_sh.transpose(2, 1, 0).reshape(E, T * BL)).astype(b16t)
        # selection matrix A^T per sequence
        ATp = np.zeros((BL, T, T), np.float32)
        oneh = np.zeros((BL, KEMB, T), np.float32)
        for j in range(BL):
            wv = wl_vals[bs][j].astype(np.int64)
            start = np.clip(t - wv, 0, None)
            denom = np.maximum(t - start, 1).astype(np.float32)
            s = np.arange(T)[:, None]
            m = (s >= start[None, :]) & (s < t[None, :])
            ATp[j] = m / denom[None, :]
            oneh[j][pos_ids[bs][j].astype(np.int64), t] = 1.0
            oneh[j][PN + wl_ids[bs][j].astype(np.int64), t] = 1.0
            oneh[j][KEMB - 1, :] = 1.0
        maskv = (t[:, None] < lengths[bs][None, :]).astype(np.float32)  # [T, BL]
        mask66 = np.broadcast_to(maskv.reshape(1, T * BL), (S, T * BL)).copy()
        in_maps.append({
            "encT_b": encT_b, "encF": encF, "ATp": ATp, "onehotT": oneh,
            "Rm": Rm, "fcw1T": fcw1T, "wihT": wihT, "whh8p": whh8,
            "l9f8p": l9f8, "zpbias": zpbias,
            "ident8": np.eye(BL, dtype=b16t),
            "combT": combT, "combB": combB,
            "mask66": mask66,
        })
    return in_maps


def kernel(**inputs):
    from concourse.bass_utils import run_bass_kernel_spmd

    if "prog" not in _PROGRAM_CACHE:
        _PROGRAM_CACHE["prog"] = _build_program()
    nc = _PROGRAM_CACHE["prog"]
    in_maps = _host_prepack(inputs)
    res = run_bass_kernel_spmd(nc, in_maps, list(range(NCORES)))
    outs = []
    for c in range(NCORES):
        o = np.asarray(res.results[c]["out"])  # [S, T*BL]
        outs.append(o.reshape(S, T, BL).transpose(2, 1, 0))  # [BL, T, S]
    return np.ascontiguousarray(np.concatenate(outs, axis=0)).astype(np.float32)


# revision 24
# speedup vs baseline: 1.2824x; 1.2824x over previous
"""Trainium2 Bass kernel for nn_Decode: masked-mean embed + fc/tanh + LSTM scan + output proj.

Strategy: pure data parallelism over batch (64 -> 8 cores x 8 sequences).
All heavy FLOPs on device; host only repacks weights and builds 0/1 index
matrices (selection matrix for the sliding-window mean, one-hot embedding
matrices, length mask) from the integer index inputs.

Key reformulations:
  - mean_emb@fc_w1^T is computed as (A @ (enc @ fc_w1^T)) where A[t,s] = 1/denom[t]
    for start[t] <= s < t, built host-side from wordlen_vals (banded 0/1 matrix).
  - embedding lookups are folded through fc: onehot @ (emb_w @ fc_w_part^T),
    with the fc bias as an extra ones-row.
  - LSTM input projections zp_t = z_t @ w_ih^T are precomputed for all t in fp8
    and injected into the per-step PSUM accumulation via a DoubleRow matmul
    whose stationary carries an identity + bias-row selector.
  - the recurrent h @ w_hh^T runs as fp8e4m3 DoubleRow matmuls (w_hh moving,
    h^T fp8 ring stationary): 256 effective contraction rows per instruction
    at 0.5 cycles/row.
  - the LSTM cell runs in bf16; tanh(c) is evaluated on the transposed c
    (PE transposes c and sigma(o) quadrants into PSUM) so h^T is produced
    directly in the fp8 ring layout the next step's matmul needs.
  - the final out = [h, enc] @ comb_w^T + comb_b runs in bf16 with the length
    mask folded into the PSUM->SBUF copy (scalar_tensor_tensor).
Gate order is permuted host-side to [i, f, o, g]; per-step issue order is
f, i, g, o so the f-gate (first consumer) completes first.
"""
import sys
import numpy as np

sys.path.insert(0, "/opt/trn_rl_repo")

B, T, E, H, LH = 64, 512, 1024, 512, 512
S = 66
PN, WN, PD, WD = 32, 8, 64, 64
NCORES = 8
BL = B // NCORES
G4 = 4 * H  # 2048
KEMB = PN + WN + 1  # 41 (pos onehot, wordlen onehot, bias row)
NZP = 2  # zp prefetch depth

_PROGRAM_CACHE = {}


def _build_program():
    from concourse import bass, tile, mybir
    from concourse import bacc

    f32 = mybir.dt.float32
    bf16 = mybir.dt.bfloat16
    f8 = mybir.dt.float8e4
    AF = mybir.ActivationFunctionType
    ALU = mybir.AluOpType
    DR = mybir.MatmulPerfMode.DoubleRow
    f32r = mybir.dt.float32r
    R = lambda ap: ap.bitcast(f32r)

    nc = bacc.Bacc("TRN2", target_bir_lowering=False)

    # ---------------- I/O ----------------
    encT_b = nc.declare_dram_parameter("encT_b", [BL, E, T], f32r, isOutput=False)
    encF = nc.declare_dram_parameter("encF", [E, T * BL], bf16, isOutput=False)
    ATp = nc.declare_dram_parameter("ATp", [BL, T, T], f32r, isOutput=False)
    onehotT = nc.declare_dram_parameter("onehotT", [BL, KEMB, T], f32r, isOutput=False)
    Rm = nc.declare_dram_parameter("Rm", [KEMB, LH], f32r, isOutput=False)
    fcw1T = nc.declare_dram_parameter("fcw1T", [E, LH], f32r, isOutput=False)
    wihT = nc.declare_dram_parameter("wihT", [LH, G4], f32r, isOutput=False)
    whh8p = nc.declare_dram_parameter("whh8p", [128, 2, 2, G4], f8, isOutput=False)
    l9f8p = nc.declare_dram_parameter("l9f8p", [8, 2, 16], f8, isOutput=False)
    zpbias = nc.declare_dram_parameter("zpbias", [8, G4], f8, isOutput=False)
    ident8 = nc.declare_dram_parameter("ident8", [BL, BL], bf16, isOutput=False)
    combT = nc.declare_dram_parameter("combT", [H + E, S], bf16, isOutput=False)
    combB = nc.declare_dram_parameter("combB", [S, 1], f32, isOutput=False)
    mask66 = nc.declare_dram_parameter("mask66", [S, T * BL], f32, isOutput=False)
    outP = nc.declare_dram_parameter("out", [S, T * BL], f32, isOutput=True)

    # ---------------- internal HBM ----------------
    zp_hbm = nc.dram_tensor("zp_hbm", [T, BL, G4], f8)

    with tile.TileContext(nc) as tc:
        # ============ persistent pools (live across whole kernel) ============
        with (
            tc.tile_pool(name="pers", bufs=1) as pers,
            tc.tile_pool(name="ring", bufs=1) as ringp,
            tc.tile_pool(name="cell", bufs=2) as cell,
        ):
            # scan weights resident: whh fp8 DoubleRow layout [p, i, j, n]
            whh8 = pers.tile([128, 2, 2, G4], f8, tag="whh8", name="whh8")
            nc.sync.dma_start(whh8[:], whh8p[:])
            l9_sb = pers.tile([8, 2, 16], f8, tag="l9", name="l9s")
            nc.sync.dma_start(l9_sb[:], l9f8p[:])
            id8_sb = pers.tile([BL, BL], bf16, tag="id8", name="id8_sb")
            nc.sync.dma_start(id8_sb[:], ident8[:])
            # zp staging tiles: [:, 0, :] = per-step zp, [:, 1, :] = bias row
            zp9 = [pers.tile([8, 2, G4], f8, tag=f"zp9_{i}", name=f"zp9_{i}") for i in range(NZP)]
            for i in range(NZP):
                nc.sync.dma_start(zp9[i][:, 1, :], zpbias[:])
            # h^T rings, fp8: [128, slot(64), q(4 = j*2+i), b(16 padded)]
            ring = [ringp.tile([128, 64, 4, 16], f8, tag=f"ring{p}", name=f"ring{p}") for p in range(2)]
            nc.vector.memset(ring[0][:].bitcast(f32), 0.0)
            nc.vector.memset(ring[1][:].bitcast(f32), 0.0)
            # LSTM cell state (bf16)
            c_sb = pers.tile([BL, H], bf16, tag="c_sb", name="c_sb")
            nc.vector.memset(c_sb[:].bitcast(f32), 0.0)

            # ================= pre-phases (per-sequence) =================
            with (
                tc.tile_pool(name="pre_w", bufs=1) as pre_w,
                tc.tile_pool(name="pre_s", bufs=1) as pre_s,
                tc.tile_pool(name="ps_pre", bufs=4, space="PSUM") as ps_pre,
            ):
                fcw1_sb = [pre_w.tile([128, LH], f32r, tag=f"fcw1_{e}", name=f"fcw1_{e}") for e in range(8)]
                for e in range(8):
                    nc.sync.dma_start(fcw1_sb[e][:], fcw1T[e * 128:(e + 1) * 128, :])
                wih_sb = [pre_w.tile([128, G4], f32r, tag=f"wih{k}", name=f"wih{k}") for k in range(4)]
                for k in range(4):
                    nc.sync.dma_start(wih_sb[k][:], wihT[k * 128:(k + 1) * 128, :])
                rm_sb = pre_w.tile([KEMB, LH], f32r, tag="rm", name="rm")
                nc.sync.dma_start(rm_sb[:], Rm[:])

                for b in range(BL):
                    # ---- load enc^T for this sequence ----
                    enc_sb = [pre_s.tile([128, T], f32r, tag=f"enc{e}", name=f"enc{e}") for e in range(8)]
                    for e in range(8):
                        nc.sync.dma_start(enc_sb[e][:], encT_b[b, e * 128:(e + 1) * 128, :])
                    # ---- P = enc @ fc_w1^T  -> [T(s), LH] ----
                    P_sb = [pre_s.tile([128, LH], f32r, tag=f"P{sc}", name=f"P{sc}") for sc in range(4)]
                    for sc in range(4):
                        ps = ps_pre.tile([128, LH], f32, tag="psA", name="psA")
                        for e in range(8):
                            nc.tensor.matmul(
                                ps[:], R(enc_sb[e][:, sc * 128:(sc + 1) * 128]),
                                R(fcw1_sb[e][:]), start=(e == 0), stop=(e == 7),
                            )
                        nc.vector.tensor_copy(P_sb[sc][:], ps[:])
                    # ---- z^T = tanh(P^T A^T + R^T onehot^T) -> [LH, T] ----
                    at_sb = [pre_s.tile([128, T], f32r, tag=f"at{sc}", name=f"at{sc}") for sc in range(4)]
                    for sc in range(4):
                        nc.sync.dma_start(at_sb[sc][:], ATp[b, sc * 128:(sc + 1) * 128, :])
                    oh_sb = pre_s.tile([KEMB, T], f32r, tag="oh", name="oh")
                    nc.sync.dma_start(oh_sb[:], onehotT[b, :, :])
                    zT_sb = [pre_s.tile([128, T], f32r, tag=f"zT{m}", name=f"zT{m}") for m in range(4)]
                    for m in range(4):
                        ps = ps_pre.tile([128, T], f32, tag="psA", name="psA2")
                        for sc in range(4):
                            nc.tensor.matmul(
                                ps[:], R(P_sb[sc][:, m * 128:(m + 1) * 128]),
                                R(at_sb[sc][:]), start=(sc == 0), stop=False,
                            )
                        nc.tensor.matmul(
                            ps[:], R(rm_sb[:, m * 128:(m + 1) * 128]), R(oh_sb[:]),
                            start=False, stop=True,
                        )
                        nc.scalar.activation(zT_sb[m][:], ps[:], AF.Tanh)
                        nc.vector.memset(zT_sb[m][:, 0:1].bitcast(f32), 0.0)  # z_0 = 0
                    # ---- zp = z @ w_ih^T -> HBM [T, b, 4H] fp8 ----
                    for mt in range(4):
                        for nb in range(4):
                            ps = ps_pre.tile([128, 512], f32, tag="psB", name="psB")
                            for kc in range(4):
                                nc.tensor.matmul(
                                    ps[:], R(zT_sb[kc][:, mt * 128:(mt + 1) * 128]),
                                    R(wih_sb[kc][:, nb * 512:(nb + 1) * 512]),
                                    start=(kc == 0), stop=(kc == 3),
                                )
                            zpo = pre_s.tile([128, 512], f8, tag="zpo", name="zpo", bufs=2)
                            nc.vector.tensor_copy(zpo[:], ps[:])
                            nc.sync.dma_start(
                                zp_hbm[mt * 128:(mt + 1) * 128, b, nb * 512:(nb + 1) * 512],
                                zpo[:],
                            )

            tc.strict_bb_all_engine_barrier()

            # ================= LSTM scan (+ fused output projection) =================
            scan_ctx = [
                tc.tile_pool(name="ps_scan", bufs=1, space="PSUM"),
                tc.tile_pool(name="ps_tp", bufs=2, space="PSUM"),
                tc.tile_pool(name="fin_w", bufs=1),
                tc.tile_pool(name="fin_s", bufs=3),
            ]
            ps_scan = scan_ctx[0].__enter__()
            ps_tp = scan_ctx[1].__enter__()
            fin_w = scan_ctx[2].__enter__()
            fin_s = scan_ctx[3].__enter__()
            cw_sb = [fin_w.tile([128, S], bf16, tag=f"cw{k}", name=f"cw{k}") for k in range(12)]
            for k in range(12):
                nc.sync.dma_start(cw_sb[k][:], combT[k * 128:(k + 1) * 128, :])
            cb_sb = fin_w.tile([S, 1], f32, tag="cb", name="cb")
            nc.sync.dma_start(cb_sb[:], combB[:])
            mk_sb = fin_w.tile([S, T * BL], f32, tag="mk", name="mk")
            nc.sync.dma_start(mk_sb[:], mask66[:])
            for i in range(NZP):
                nc.sync.dma_start(zp9[i][:, 0, :], zp_hbm[i])

            # gate banks (host perm [f, i, g, o]): bank nb = cols [nb*512, ...).
            # All sigmoids are tanh half-angle: sigma(x) = 0.5*tanh(x/2) + 0.5,
            # with the 0.5 input scales folded into w_ih/w_hh/bias host-side,
            # and h' = 2h (folded into w_hh rows and comb_w rows).
            BANK_ORDER = [0, 1, 2, 3]  # f, i, g, o

            def inject(t, nb):
                pg = ps_scan.tile([128, 512], f32, tag="gates", name="pgb", bufs=5)
                nc.tensor.matmul(
                    pg[0:16, :], l9_sb[:],
                    zp9[t % NZP][:, :, nb * 512:(nb + 1) * 512],
                    start=True, stop=False, perf_mode=DR,
                )
                return pg

            psb = {nb: inject(0, nb) for nb in BANK_ORDER}
            for t in range(T):
                par = (t // 64) % 2
                slot = t % 64
                pprev = ((t - 1) // 64) % 2
                sprev = (t - 1) % 64

                # recurrent fp8 DoubleRow matmuls for step t (f first)
                for nb in BANK_ORDER:
                    for j in range(2):
                        nc.tensor.matmul(
                            psb[nb][0:16, :],
                            ring[pprev][:, sprev, 2 * j:2 * j + 2, :],
                            whh8[:, :, j, nb * 512:(nb + 1) * 512],
                            start=False, stop=(j == 1), perf_mode=DR,
                        )

                # PE p-state fillers: keep the array streaming while the cell
                # chain runs (PE drops to low clock on idle gaps otherwise).
                dmy = ps_scan.tile([128, 512], f32, tag="gates", name="dmy", bufs=5)
                for _ in range(5):
                    nc.tensor.matmul(
                        dmy[0:16, :],
                        ring[pprev][:, sprev, 0:2, :],
                        whh8[:, :, 0, 0:512],
                        start=True, stop=True, perf_mode=DR,
                    )

                # tanh-only cell, bf16:
                #   cs' = 2c;  m2t = (tf+1)*cs';  m1 = (ti+1)*tg
                #   cs'_new = 0.5*m2t + m1;  h' = (to^T+1)*tanh(0.5*cs')^T
                tf = cell.tile([BL, H], bf16, tag="tf", name="tf")
                ti = cell.tile([BL, H], bf16, tag="ti", name="ti")
                tg = cell.tile([BL, H], bf16, tag="tg", name="tg")
                to = cell.tile([BL, H], bf16, tag="to", name="to")
                m1 = cell.tile([BL, H], bf16, tag="m1", name="m1")
                m2t = cell.tile([BL, H], bf16, tag="m2t", name="m2t")
                tcT = cell.tile([128, 4, 8], bf16, tag="tcT", name="tcT")
                nc.scalar.activation(tf[:], psb[0][0:8, :], AF.Tanh)
                nc.scalar.activation(ti[:], psb[1][0:8, :], AF.Tanh)
                nc.scalar.activation(tg[:], psb[2][0:8, :], AF.Tanh)
                nc.scalar.activation(to[:], psb[3][0:8, :], AF.Tanh)
                nc.vector.scalar_tensor_tensor(
                    m2t[:], tf[:], 1.0, c_sb[:], op0=ALU.add, op1=ALU.mult)
                nc.vector.scalar_tensor_tensor(
                    m1[:], ti[:], 1.0, tg[:], op0=ALU.add, op1=ALU.mult)
                nc.vector.scalar_tensor_tensor(
                    c_sb[:], m2t[:], 0.5, m1[:], op0=ALU.mult, op1=ALU.add)
                psb_next = {}
                if t + 1 < T:
                    psb_next[0] = inject(t + 1, 0)
                    psb_next[1] = inject(t + 1, 1)

                # transposed tail
                tp = ps_tp.tile([128, 2, 4, 8], bf16, tag="tp", name="tp")
                for q in range(4):
                    nc.tensor.transpose(
                        tp[:, 0, q, :],
                        c_sb[:, q * 128:(q + 1) * 128], id8_sb[:],
                    )
                if t + 1 < T:
                    psb_next[2] = inject(t + 1, 2)
                for q in range(4):
                    nc.tensor.transpose(
                        tp[:, 1, q, :],
                        to[:, q * 128:(q + 1) * 128], id8_sb[:],
                    )
                if t + 1 < T:
                    psb_next[3] = inject(t + 1, 3)
                nc.scalar.activation(tcT[:], tp[:, 0, :, :], AF.Tanh, scale=0.5)
                nc.vector.scalar_tensor_tensor(
                    ring[par][:, slot, :, 0:8],
                    tp[:, 1, :, :], 1.0, tcT[:], op0=ALU.add, op1=ALU.mult)

                if slot == 63:
                    ncol = t // 64
                    cols = slice(ncol * 512, (ncol + 1) * 512)
                    rb = fin_s.tile([128, 4, 64, 8], bf16, tag="rb", name="rb")
                    for q in range(4):
                        nc.vector.tensor_copy(
                            rb[:, q, :, :], ring[par][:, :, q, 0:8],
                        )
                    psf = ps_tp.tile([S, 512], f32, tag="psF", name="psF", bufs=1)
                    for q in range(4):
                        nc.tensor.matmul(
                            psf[:], cw_sb[q][:], rb[:, q, :, :],
                            start=(q == 0), stop=False,
                        )
                    for ke in range(8):
                        re = fin_s.tile([128, 512], bf16, tag="re", name="re")
                        nc.sync.dma_start(re[:], encF[ke * 128:(ke + 1) * 128, cols])
                        nc.tensor.matmul(
                            psf[:], cw_sb[4 + ke][:], re[:], start=False, stop=(ke == 7),
                        )
                    ob = fin_s.tile([S, 512], f32, tag="ob", name="ob")
                    nc.vector.scalar_tensor_tensor(
                        ob[:], psf[:], cb_sb[:, 0:1], mk_sb[:, cols],
                        op0=ALU.add, op1=ALU.mult,
                    )
                    if ncol == 0:
                        nc.vector.memset(ob[0:1, 0:8], -1e30)
                    nc.sync.dma_start(outP[:, cols], ob[:])
                if t + NZP < T:
                    nc.sync.dma_start(zp9[t % NZP][:, 0, :], zp_hbm[t + NZP])
                psb = psb_next

            for c in reversed(scan_ctx):
                c.__exit__(None, None, None)

    nc.compile()
    return nc


def _host_prepack(inputs):
    """Build per-core in_maps from full inputs. Index plumbing + weight repacking only."""
    import ml_dtypes
    f8t = ml_dtypes.float8_e4m3
    b16t = ml_dtypes.bfloat16
    enc = np.asarray(inputs["encode_out"], np.float32)
    pos_emb_w = np.asarray(inputs["pos_emb_w"], np.float32)
    wl_emb_w = np.asarray(inputs["wordlen_emb_w"], np.float32)
    fc_w = np.asarray(inputs["fc_w"], np.float32)
    fc_b = np.asarray(inputs["fc_b"], np.float32)
    w_ih = np.asarray(inputs["w_ih"], np.float32)
    w_hh = np.asarray(inputs["w_hh"], np.float32)
    b_ih = np.asarray(inputs["b_ih"], np.float32)
    b_hh = np.asarray(inputs["b_hh"], np.float32)
    comb_w = np.asarray(inputs["comb_w"], np.float32)
    comb_b = np.asarray(inputs["comb_b"], np.float32)
    pos_ids = np.asarray(inputs["pos_ids"])
    wl_ids = np.asarray(inputs["wordlen_ids"])
    wl_vals = np.asarray(inputs["wordlen_vals"])
    lengths = np.asarray(inputs["lengths"])

    t = np.arange(T)
    # gate permutation [f, i, g, o]; tanh half-angle scales: f,i,o inputs x0.5
    perm = np.r_[H:2 * H, 0:H, 2 * H:3 * H, 3 * H:4 * H]
    colscale = np.concatenate(
        [np.full(H, 0.5), np.full(H, 0.5), np.ones(H), np.full(H, 0.5)]
    ).astype(np.float32)
    wihT = np.ascontiguousarray(w_ih[perm].T) * colscale[None, :]  # [LH, 4H]
    # w_hh rows x0.5 compensates h' = 2h
    whhT = np.ascontiguousarray(w_hh[perm].T) * colscale[None, :] * 0.5
    # fp8 DoubleRow layout [p, i, j, n]: row K = j*256 + i*128 + p
    whh8 = np.ascontiguousarray(
        whhT.reshape(2, 2, 128, G4).transpose(2, 1, 0, 3)).astype(f8t)
    # inject stationary: [p, 0, m] = identity; [0, 1, m] = 1 (bias row)
    l9f8 = np.zeros((8, 2, 16), np.float32)
    l9f8[:, 0, :8] = np.eye(8)
    l9f8[0, 1, :8] = 1.0
    l9f8 = l9f8.astype(f8t)
    # bias row for zp9[:, 1, :]: partition 0 = b_ih + b_hh, rest 0
    zpbias = np.zeros((8, G4), np.float32)
    zpbias[0] = (b_ih + b_hh)[perm] * colscale
    zpbias = zpbias.astype(f8t)
    # embedding folding
    fc_w1 = fc_w[:, :E]
    fc_w2 = fc_w[:, E:E + WD]
    fc_w3 = fc_w[:, E + WD:]
    Rm = np.concatenate(
        [pos_emb_w @ fc_w3.T, wl_emb_w @ fc_w2.T, fc_b[None, :]], axis=0
    ).astype(np.float32)  # [41, LH]
    fcw1T = np.ascontiguousarray(fc_w1.T)  # [E, LH]
    combT = np.ascontiguousarray(comb_w.T)  # [1536, S]
    combT[:H] *= 0.5  # h' = 2h compensation
    combT = combT.astype(b16t)
    combB = comb_b[:, None].astype(np.float32)

    in_maps = []
    for c in range(NCORES):
        bs = slice(c * BL, (c + 1) * BL)
        enc_sh = enc[bs]  # [BL, T, E]
        encT_b = np.ascontiguousarray(enc_sh.transpose(0, 2, 1))  # [BL, E, T]
        encF = np.ascontiguousarray(
            enc_sh.transpose(2, 1, 0).reshape(E, T * BL)).astype(b16t)
        # selection matrix A^T per sequence
        ATp = np.zeros((BL, T, T), np.float32)
        oneh = np.zeros((BL, KEMB, T), np.float32)
        for j in range(BL):
            wv = wl_vals[bs][j].astype(np.int64)
            start = np.clip(t - wv, 0, None)
            denom = np.maximum(t - start, 1).astype(np.float32)
            s = np.arange(T)[:, None]
            m = (s >= start[None, :]) & (s < t[None, :])
            ATp[j] = m / denom[None, :]
            oneh[j][pos_ids[bs][j].astype(np.int64), t] = 1.0
            oneh[j][PN + wl_ids[bs][j].astype(np.int64), t] = 1.0
            oneh[j][KEMB - 1, :] = 1.0
        maskv = (t[:, None] < lengths[bs][None, :]).astype(np.float32)  # [T, BL]
        mask66 = np.broadcast_to(maskv.reshape(1, T * BL), (S, T * BL)).copy()
        in_maps.append({
            "encT_b": encT_b, "encF": encF, "ATp": ATp, "onehotT": oneh,
            "Rm": Rm, "fcw1T": fcw1T, "wihT": wihT, "whh8p": whh8,
            "l9f8p": l9f8, "zpbias": zpbias,
            "ident8": np.eye(BL, dtype=b16t),
            "combT": combT, "combB": combB,
            "mask66": mask66,
        })
    return in_maps


def kernel(**inputs):
    from concourse.bass_utils import run_bass_kernel_spmd

    if "prog" not in _PROGRAM_CACHE:
        _PROGRAM_CACHE["prog"] = _build_program()
    nc = _PROGRAM_CACHE["prog"]
    in_maps = _host_prepack(inputs)
    res = run_bass_kernel_spmd(nc, in_maps, list(range(NCORES)))
    outs = []
    for c in range(NCORES):
        o = np.asarray(res.results[c]["out"])  # [S, T*BL]
        outs.append(o.reshape(S, T, BL).transpose(2, 1, 0))  # [BL, T, S]
    return np.ascontiguousarray(np.concatenate(outs, axis=0)).astype(np.float32)


# revision 25
# speedup vs baseline: 1.2926x; 1.0080x over previous
"""Trainium2 Bass kernel for nn_Decode: masked-mean embed + fc/tanh + LSTM scan + output proj.

Strategy: pure data parallelism over batch (64 -> 8 cores x 8 sequences).
All heavy FLOPs on device; host only repacks weights and builds 0/1 index
matrices (selection matrix for the sliding-window mean, one-hot embedding
matrices, length mask) from the integer index inputs.

Key reformulations:
  - mean_emb@fc_w1^T is computed as (A @ (enc @ fc_w1^T)) where A[t,s] = 1/denom[t]
    for start[t] <= s < t, built host-side from wordlen_vals (banded 0/1 matrix).
  - embedding lookups are folded through fc: onehot @ (emb_w @ fc_w_part^T),
    with the fc bias as an extra ones-row.
  - LSTM input projections zp_t = z_t @ w_ih^T are precomputed for all t in fp8
    and injected into the per-step PSUM accumulation via a DoubleRow matmul
    whose stationary carries an identity + bias-row selector.
  - the recurrent h @ w_hh^T runs as fp8e4m3 DoubleRow matmuls (w_hh moving,
    h^T fp8 ring stationary): 256 effective contraction rows per instruction
    at 0.5 cycles/row.
  - the LSTM cell runs in bf16; tanh(c) is evaluated on the transposed c
    (PE transposes c and sigma(o) quadrants into PSUM) so h^T is produced
    directly in the fp8 ring layout the next step's matmul needs.
  - the final out = [h, enc] @ comb_w^T + comb_b runs in bf16 with the length
    mask folded into the PSUM->SBUF copy (scalar_tensor_tensor).
Gate order is permuted host-side to [i, f, o, g]; per-step issue order is
f, i, g, o so the f-gate (first consumer) completes first.
"""
import sys
import numpy as np

sys.path.insert(0, "/opt/trn_rl_repo")

B, T, E, H, LH = 64, 512, 1024, 512, 512
S = 66
PN, WN, PD, WD = 32, 8, 64, 64
NCORES = 8
BL = B // NCORES
G4 = 4 * H  # 2048
KEMB = PN + WN + 1  # 41 (pos onehot, wordlen onehot, bias row)
NZP = 2  # zp prefetch depth

_PROGRAM_CACHE = {}


def _build_program():
    from concourse import bass, tile, mybir
    from concourse import bacc

    f32 = mybir.dt.float32
    bf16 = mybir.dt.bfloat16
    f8 = mybir.dt.float8e4
    AF = mybir.ActivationFunctionType
    ALU = mybir.AluOpType
    DR = mybir.MatmulPerfMode.DoubleRow
    f32r = mybir.dt.float32r
    R = lambda ap: ap.bitcast(f32r)

    nc = bacc.Bacc("TRN2", target_bir_lowering=False)

    # ---------------- I/O ----------------
    encT_b = nc.declare_dram_parameter("encT_b", [BL, E, T], f32r, isOutput=False)
    encF = nc.declare_dram_parameter("encF", [E, T * BL], bf16, isOutput=False)
    ATp = nc.declare_dram_parameter("ATp", [BL, T, T], f32r, isOutput=False)
    onehotT = nc.declare_dram_parameter("onehotT", [BL, KEMB, T], f32r, isOutput=False)
    Rm = nc.declare_dram_parameter("Rm", [KEMB, LH], f32r, isOutput=False)
    fcw1T = nc.declare_dram_parameter("fcw1T", [E, LH], f32r, isOutput=False)
    wihT = nc.declare_dram_parameter("wihT", [LH, G4], f32r, isOutput=False)
    whh8p = nc.declare_dram_parameter("whh8p", [128, 2, 2, G4], f8, isOutput=False)
    l9f8p = nc.declare_dram_parameter("l9f8p", [8, 2, 16], f8, isOutput=False)
    zpbias = nc.declare_dram_parameter("zpbias", [8, G4], f8, isOutput=False)
    ident8 = nc.declare_dram_parameter("ident8", [BL, BL], bf16, isOutput=False)
    combT = nc.declare_dram_parameter("combT", [H + E, S], bf16, isOutput=False)
    combB = nc.declare_dram_parameter("combB", [S, 1], f32, isOutput=False)
    mask66 = nc.declare_dram_parameter("mask66", [S, T * BL], f32, isOutput=False)
    outP = nc.declare_dram_parameter("out", [S, T * BL], f32, isOutput=True)

    # ---------------- internal HBM ----------------
    zp_hbm = nc.dram_tensor("zp_hbm", [T, BL, G4], f8)

    with tile.TileContext(nc) as tc:
        # ============ persistent pools (live across whole kernel) ============
        with (
            tc.tile_pool(name="pers", bufs=1) as pers,
            tc.tile_pool(name="ring", bufs=1) as ringp,
            tc.tile_pool(name="cell", bufs=2) as cell,
        ):
            # scan weights resident: whh fp8 DoubleRow layout [p, i, j, n]
            whh8 = pers.tile([128, 2, 2, G4], f8, tag="whh8", name="whh8")
            nc.sync.dma_start(whh8[:], whh8p[:])
            l9_sb = pers.tile([8, 2, 16], f8, tag="l9", name="l9s")
            nc.sync.dma_start(l9_sb[:], l9f8p[:])
            id8_sb = pers.tile([BL, BL], bf16, tag="id8", name="id8_sb")
            nc.sync.dma_start(id8_sb[:], ident8[:])
            # zp staging tiles: [:, 0, :] = per-step zp, [:, 1, :] = bias row
            zp9 = [pers.tile([8, 2, G4], f8, tag=f"zp9_{i}", name=f"zp9_{i}") for i in range(NZP)]
            for i in range(NZP):
                nc.sync.dma_start(zp9[i][:, 1, :], zpbias[:])
            # h^T rings, fp8: [128, slot(64), q(4 = j*2+i), b(16 padded)]
            ring = [ringp.tile([128, 64, 4, 16], f8, tag=f"ring{p}", name=f"ring{p}") for p in range(2)]
            nc.vector.memset(ring[0][:].bitcast(f32), 0.0)
            nc.vector.memset(ring[1][:].bitcast(f32), 0.0)
            # LSTM cell state (bf16)
            c_sb = pers.tile([BL, H], bf16, tag="c_sb", name="c_sb")
            nc.vector.memset(c_sb[:].bitcast(f32), 0.0)

            # ================= pre-phases (per-sequence) =================
            with (
                tc.tile_pool(name="pre_w", bufs=1) as pre_w,
                tc.tile_pool(name="pre_s", bufs=1) as pre_s,
                tc.tile_pool(name="ps_pre", bufs=4, space="PSUM") as ps_pre,
            ):
                fcw1_sb = [pre_w.tile([128, LH], f32r, tag=f"fcw1_{e}", name=f"fcw1_{e}") for e in range(8)]
                for e in range(8):
                    nc.sync.dma_start(fcw1_sb[e][:], fcw1T[e * 128:(e + 1) * 128, :])
                wih_sb = [pre_w.tile([128, G4], f32r, tag=f"wih{k}", name=f"wih{k}") for k in range(4)]
                for k in range(4):
                    nc.sync.dma_start(wih_sb[k][:], wihT[k * 128:(k + 1) * 128, :])
                rm_sb = pre_w.tile([KEMB, LH], f32r, tag="rm", name="rm")
                nc.sync.dma_start(rm_sb[:], Rm[:])

                for b in range(BL):
                    # ---- load enc^T for this sequence ----
                    enc_sb = [pre_s.tile([128, T], f32r, tag=f"enc{e}", name=f"enc{e}") for e in range(8)]
                    for e in range(8):
                        nc.sync.dma_start(enc_sb[e][:], encT_b[b, e * 128:(e + 1) * 128, :])
                    # ---- P = enc @ fc_w1^T  -> [T(s), LH] ----
                    P_sb = [pre_s.tile([128, LH], f32r, tag=f"P{sc}", name=f"P{sc}") for sc in range(4)]
                    for sc in range(4):
                        ps = ps_pre.tile([128, LH], f32, tag="psA", name="psA")
                        for e in range(8):
                            nc.tensor.matmul(
                                ps[:], R(enc_sb[e][:, sc * 128:(sc + 1) * 128]),
                                R(fcw1_sb[e][:]), start=(e == 0), stop=(e == 7),
                            )
                        nc.vector.tensor_copy(P_sb[sc][:], ps[:])
                    # ---- z^T = tanh(P^T A^T + R^T onehot^T) -> [LH, T] ----
                    at_sb = [pre_s.tile([128, T], f32r, tag=f"at{sc}", name=f"at{sc}") for sc in range(4)]
                    for sc in range(4):
                        nc.sync.dma_start(at_sb[sc][:], ATp[b, sc * 128:(sc + 1) * 128, :])
                    oh_sb = pre_s.tile([KEMB, T], f32r, tag="oh", name="oh")
                    nc.sync.dma_start(oh_sb[:], onehotT[b, :, :])
                    zT_sb = [pre_s.tile([128, T], f32r, tag=f"zT{m}", name=f"zT{m}") for m in range(4)]
                    for m in range(4):
                        ps = ps_pre.tile([128, T], f32, tag="psA", name="psA2")
                        for sc in range(4):
                            nc.tensor.matmul(
                                ps[:], R(P_sb[sc][:, m * 128:(m + 1) * 128]),
                                R(at_sb[sc][:]), start=(sc == 0), stop=False,
                            )
                        nc.tensor.matmul(
                            ps[:], R(rm_sb[:, m * 128:(m + 1) * 128]), R(oh_sb[:]),
                            start=False, stop=True,
                        )
                        nc.scalar.activation(zT_sb[m][:], ps[:], AF.Tanh)
                        nc.vector.memset(zT_sb[m][:, 0:1].bitcast(f32), 0.0)  # z_0 = 0
                    # ---- zp = z @ w_ih^T -> HBM [T, b, 4H] fp8 ----
                    for mt in range(4):
                        for nb in range(4):
                            ps = ps_pre.tile([128, 512], f32, tag="psB", name="psB")
                            for kc in range(4):
                                nc.tensor.matmul(
                                    ps[:], R(zT_sb[kc][:, mt * 128:(mt + 1) * 128]),
                                    R(wih_sb[kc][:, nb * 512:(nb + 1) * 512]),
                                    start=(kc == 0), stop=(kc == 3),
                                )
                            zpo = pre_s.tile([128, 512], f8, tag="zpo", name="zpo", bufs=2)
                            nc.vector.tensor_copy(zpo[:], ps[:])
                            nc.sync.dma_start(
                                zp_hbm[mt * 128:(mt + 1) * 128, b, nb * 512:(nb + 1) * 512],
                                zpo[:],
                            )

            tc.strict_bb_all_engine_barrier()

            # ================= LSTM scan (+ fused output projection) =================
            scan_ctx = [
                tc.tile_pool(name="ps_scan", bufs=1, space="PSUM"),
                tc.tile_pool(name="ps_tp", bufs=2, space="PSUM"),
                tc.tile_pool(name="fin_w", bufs=1),
                tc.tile_pool(name="fin_s", bufs=3),
            ]
            ps_scan = scan_ctx[0].__enter__()
            ps_tp = scan_ctx[1].__enter__()
            fin_w = scan_ctx[2].__enter__()
            fin_s = scan_ctx[3].__enter__()
            cw_sb = [fin_w.tile([128, S], bf16, tag=f"cw{k}", name=f"cw{k}") for k in range(12)]
            for k in range(12):
                nc.sync.dma_start(cw_sb[k][:], combT[k * 128:(k + 1) * 128, :])
            cb_sb = fin_w.tile([S, 1], f32, tag="cb", name="cb")
            nc.sync.dma_start(cb_sb[:], combB[:])
            mk_sb = fin_w.tile([S, T * BL], f32, tag="mk", name="mk")
            nc.sync.dma_start(mk_sb[:], mask66[:])
            for i in range(NZP):
                nc.sync.dma_start(zp9[i][:, 0, :], zp_hbm[i])

            # gate banks (host perm [f, i, g, o]): bank nb = cols [nb*512, ...).
            # All sigmoids are tanh half-angle: sigma(x) = 0.5*tanh(x/2) + 0.5,
            # with the 0.5 input scales folded into w_ih/w_hh/bias host-side,
            # and h' = 2h (folded into w_hh rows and comb_w rows).
            BANK_ORDER = [0, 1, 2, 3]  # f, i, g, o

            def inject(t, nb):
                pg = ps_scan.tile([128, 512], f32, tag="gates", name="pgb", bufs=5)
                nc.tensor.matmul(
                    pg[0:16, :], l9_sb[:],
                    zp9[t % NZP][:, :, nb * 512:(nb + 1) * 512],
                    start=True, stop=False, perf_mode=DR,
                )
                return pg

            psb = {nb: inject(0, nb) for nb in BANK_ORDER}
            for t in range(T):
                par = (t // 64) % 2
                slot = t % 64
                pprev = ((t - 1) // 64) % 2
                sprev = (t - 1) % 64

                # recurrent fp8 DoubleRow matmuls for step t (f first)
                for nb in BANK_ORDER:
                    for j in range(2):
                        nc.tensor.matmul(
                            psb[nb][0:16, :],
                            ring[pprev][:, sprev, 2 * j:2 * j + 2, :],
                            whh8[:, :, j, nb * 512:(nb + 1) * 512],
                            start=False, stop=(j == 1), perf_mode=DR,
                        )

                # PE p-state fillers: keep the array streaming while the cell
                # chain runs (PE drops to low clock on idle gaps otherwise).
                # Short 256-col fillers: similar duty to 5x512 but the worst-
                # case drain in front of the chain-critical c-transposes is
                # a quarter as long.
                dmy = ps_scan.tile([128, 512], f32, tag="gates", name="dmy", bufs=5)
                for _ in range(8):
                    nc.tensor.matmul(
                        dmy[0:16, 0:256],
                        ring[pprev][:, sprev, 0:2, :],
                        whh8[:, :, 0, 0:256],
                        start=True, stop=True, perf_mode=DR,
                    )

                # tanh-only cell, bf16:
                #   cs' = 2c;  m2t = (tf+1)*cs';  m1 = (ti+1)*tg
                #   cs'_new = 0.5*m2t + m1;  h' = (to^T+1)*tanh(0.5*cs')^T
                tf = cell.tile([BL, H], bf16, tag="tf", name="tf")
                ti = cell.tile([BL, H], bf16, tag="ti", name="ti")
                tg = cell.tile([BL, H], bf16, tag="tg", name="tg")
                to = cell.tile([BL, H], bf16, tag="to", name="to")
                m1 = cell.tile([BL, H], bf16, tag="m1", name="m1")
                m2t = cell.tile([BL, H], bf16, tag="m2t", name="m2t")
                tcT = cell.tile([128, 4, 8], bf16, tag="tcT", name="tcT")
                nc.scalar.activation(tf[:], psb[0][0:8, :], AF.Tanh)
                nc.scalar.activation(ti[:], psb[1][0:8, :], AF.Tanh)
                nc.scalar.activation(tg[:], psb[2][0:8, :], AF.Tanh)
                nc.scalar.activation(to[:], psb[3][0:8, :], AF.Tanh)
                nc.vector.scalar_tensor_tensor(
                    m2t[:], tf[:], 1.0, c_sb[:], op0=ALU.add, op1=ALU.mult)
                nc.vector.scalar_tensor_tensor(
                    m1[:], ti[:], 1.0, tg[:], op0=ALU.add, op1=ALU.mult)
                nc.vector.scalar_tensor_tensor(
                    c_sb[:], m2t[:], 0.5, m1[:], op0=ALU.mult, op1=ALU.add)
                psb_next = {}
                if t + 1 < T:
                    psb_next[0] = inject(t + 1, 0)
                    psb_next[1] = inject(t + 1, 1)

                # transposed tail
                tp = ps_tp.tile([128, 2, 4, 8], bf16, tag="tp", name="tp")
                for q in range(4):
                    nc.tensor.transpose(
                        tp[:, 0, q, :],
                        c_sb[:, q * 128:(q + 1) * 128], id8_sb[:],
                    )
                if t + 1 < T:
                    psb_next[2] = inject(t + 1, 2)
                for q in range(4):
                    nc.tensor.transpose(
                        tp[:, 1, q, :],
                        to[:, q * 128:(q + 1) * 128], id8_sb[:],
                    )
                if t + 1 < T:
                    psb_next[3] = inject(t + 1, 3)
                nc.scalar.activation(tcT[:], tp[:, 0, :, :], AF.Tanh, scale=0.5)
                nc.vector.scalar_tensor_tensor(
                    ring[par][:, slot, :, 0:8],
                    tp[:, 1, :, :], 1.0, tcT[:], op0=ALU.add, op1=ALU.mult)

                if slot == 63:
                    ncol = t // 64
                    cols = slice(ncol * 512, (ncol + 1) * 512)
                    rb = fin_s.tile([128, 4, 64, 8], bf16, tag="rb", name="rb")
                    for q in range(4):
                        nc.vector.tensor_copy(
                            rb[:, q, :, :], ring[par][:, :, q, 0:8],
                        )
                    psf = ps_tp.tile([S, 512], f32, tag="psF", name="psF", bufs=1)
                    for q in range(4):
                        nc.tensor.matmul(
                            psf[:], cw_sb[q][:], rb[:, q, :, :],
                            start=(q == 0), stop=False,
                        )
                    for ke in range(8):
                        re = fin_s.tile([128, 512], bf16, tag="re", name="re")
                        nc.sync.dma_start(re[:], encF[ke * 128:(ke + 1) * 128, cols])
                        nc.tensor.matmul(
                            psf[:], cw_sb[4 + ke][:], re[:], start=False, stop=(ke == 7),
                        )
                    ob = fin_s.tile([S, 512], f32, tag="ob", name="ob")
                    nc.vector.scalar_tensor_tensor(
                        ob[:], psf[:], cb_sb[:, 0:1], mk_sb[:, cols],
                        op0=ALU.add, op1=ALU.mult,
                    )
                    if ncol == 0:
                        nc.vector.memset(ob[0:1, 0:8], -1e30)
                    nc.sync.dma_start(outP[:, cols], ob[:])
                if t + NZP < T:
                    nc.sync.dma_start(zp9[t % NZP][:, 0, :], zp_hbm[t + NZP])
                psb = psb_next

            for c in reversed(scan_ctx):
                c.__exit__(None, None, None)

    nc.compile()
    return nc


def _host_prepack(inputs):
    """Build per-core in_maps from full inputs. Index plumbing + weight repacking only."""
    import ml_dtypes
    f8t = ml_dtypes.float8_e4m3
    b16t = ml_dtypes.bfloat16
    enc = np.asarray(inputs["encode_out"], np.float32)
    pos_emb_w = np.asarray(inputs["pos_emb_w"], np.float32)
    wl_emb_w = np.asarray(inputs["wordlen_emb_w"], np.float32)
    fc_w = np.asarray(inputs["fc_w"], np.float32)
    fc_b = np.asarray(inputs["fc_b"], np.float32)
    w_ih = np.asarray(inputs["w_ih"], np.float32)
    w_hh = np.asarray(inputs["w_hh"], np.float32)
    b_ih = np.asarray(inputs["b_ih"], np.float32)
    b_hh = np.asarray(inputs["b_hh"], np.float32)
    comb_w = np.asarray(inputs["comb_w"], np.float32)
    comb_b = np.asarray(inputs["comb_b"], np.float32)
    pos_ids = np.asarray(inputs["pos_ids"])
    wl_ids = np.asarray(inputs["wordlen_ids"])
    wl_vals = np.asarray(inputs["wordlen_vals"])
    lengths = np.asarray(inputs["lengths"])

    t = np.arange(T)
    # gate permutation [f, i, g, o]; tanh half-angle scales: f,i,o inputs x0.5
    perm = np.r_[H:2 * H, 0:H, 2 * H:3 * H, 3 * H:4 * H]
    colscale = np.concatenate(
        [np.full(H, 0.5), np.full(H, 0.5), np.ones(H), np.full(H, 0.5)]
    ).astype(np.float32)
    wihT = np.ascontiguousarray(w_ih[perm].T) * colscale[None, :]  # [LH, 4H]
    # w_hh rows x0.5 compensates h' = 2h
    whhT = np.ascontiguousarray(w_hh[perm].T) * colscale[None, :] * 0.5
    # fp8 DoubleRow layout [p, i, j, n]: row K = j*256 + i*128 + p
    whh8 = np.ascontiguousarray(
        whhT.reshape(2, 2, 128, G4).transpose(2, 1, 0, 3)).astype(f8t)
    # inject stationary: [p, 0, m] = identity; [0, 1, m] = 1 (bias row)
    l9f8 = np.zeros((8, 2, 16), np.float32)
    l9f8[:, 0, :8] = np.eye(8)
    l9f8[0, 1, :8] = 1.0
    l9f8 = l9f8.astype(f8t)
    # bias row for zp9[:, 1, :]: partition 0 = b_ih + b_hh, rest 0
    zpbias = np.zeros((8, G4), np.float32)
    zpbias[0] = (b_ih + b_hh)[perm] * colscale
    zpbias = zpbias.astype(f8t)
    # embedding folding
    fc_w1 = fc_w[:, :E]
    fc_w2 = fc_w[:, E:E + WD]
    fc_w3 = fc_w[:, E + WD:]
    Rm = np.concatenate(
        [pos_emb_w @ fc_w3.T, wl_emb_w @ fc_w2.T, fc_b[None, :]], axis=0
    ).astype(np.float32)  # [41, LH]
    fcw1T = np.ascontiguousarray(fc_w1.T)  # [E, LH]
    combT = np.ascontiguousarray(comb_w.T)  # [1536, S]
    combT[:H] *= 0.5  # h' = 2h compensation
    combT = combT.astype(b16t)
    combB = comb_b[:, None].astype(np.float32)

    in_maps = []
    for c in range(NCORES):
        bs = slice(c * BL, (c + 1) * BL)
        enc_sh = enc[bs]  # [BL, T, E]
        encT_b = np.ascontiguousarray(enc_sh.transpose(0, 2, 1))  # [BL, E, T]
        encF = np.ascontiguousarray(
            enc_sh.transpose(2, 1, 0).reshape(E, T * BL)).astype(b16t)
        # selection matrix A^T per sequence
        ATp = np.zeros((BL, T, T), np.float32)
        oneh = np.zeros((BL, KEMB, T), np.float32)
        for j in range(BL):
            wv = wl_vals[bs][j].astype(np.int64)
            start = np.clip(t - wv, 0, None)
            denom = np.maximum(t - start, 1).astype(np.float32)
            s = np.arange(T)[:, None]
            m = (s >= start[None, :]) & (s < t[None, :])
            ATp[j] = m / denom[None, :]
            oneh[j][pos_ids[bs][j].astype(np.int64), t] = 1.0
            oneh[j][PN + wl_ids[bs][j].astype(np.int64), t] = 1.0
            oneh[j][KEMB - 1, :] = 1.0
        maskv = (t[:, None] < lengths[bs][None, :]).astype(np.float32)  # [T, BL]
        mask66 = np.broadcast_to(maskv.reshape(1, T * BL), (S, T * BL)).copy()
        in_maps.append({
            "encT_b": encT_b, "encF": encF, "ATp": ATp, "onehotT": oneh,
            "Rm": Rm, "fcw1T": fcw1T, "wihT": wihT, "whh8p": whh8,
            "l9f8p": l9f8, "zpbias": zpbias,
            "ident8": np.eye(BL, dtype=b16t),
            "combT": combT, "combB": combB,
            "mask66": mask66,
        })
    return in_maps


def kernel(**inputs):
    from concourse.bass_utils import run_bass_kernel_spmd

    if "prog" not in _PROGRAM_CACHE:
        _PROGRAM_CACHE["prog"] = _build_program()
    nc = _PROGRAM_CACHE["prog"]
    in_maps = _host_prepack(inputs)
    res = run_bass_kernel_spmd(nc, in_maps, list(range(NCORES)))
    outs = []
    for c in range(NCORES):
        o = np.asarray(res.results[c]["out"])  # [S, T*BL]
        outs.append(o.reshape(S, T, BL).transpose(2, 1, 0))  # [BL, T, S]
    return np.ascontiguousarray(np.concatenate(outs, axis=0)).astype(np.float32)


# revision 26
# speedup vs baseline: 1.2961x; 1.0027x over previous
"""Trainium2 Bass kernel for nn_Decode: masked-mean embed + fc/tanh + LSTM scan + output proj.

Strategy: pure data parallelism over batch (64 -> 8 cores x 8 sequences).
All heavy FLOPs on device; host only repacks weights and builds 0/1 index
matrices (selection matrix for the sliding-window mean, one-hot embedding
matrices, length mask) from the integer index inputs.

Key reformulations:
  - mean_emb@fc_w1^T is computed as (A @ (enc @ fc_w1^T)) where A[t,s] = 1/denom[t]
    for start[t] <= s < t, built host-side from wordlen_vals (banded 0/1 matrix).
  - embedding lookups are folded through fc: onehot @ (emb_w @ fc_w_part^T),
    with the fc bias as an extra ones-row.
  - LSTM input projections zp_t = z_t @ w_ih^T are precomputed for all t in fp8
    and injected into the per-step PSUM accumulation via a DoubleRow matmul
    whose stationary carries an identity + bias-row selector.
  - the recurrent h @ w_hh^T runs as fp8e4m3 DoubleRow matmuls (w_hh moving,
    h^T fp8 ring stationary): 256 effective contraction rows per instruction
    at 0.5 cycles/row.
  - the LSTM cell runs in bf16; tanh(c) is evaluated on the transposed c
    (PE transposes c and sigma(o) quadrants into PSUM) so h^T is produced
    directly in the fp8 ring layout the next step's matmul needs.
  - the final out = [h, enc] @ comb_w^T + comb_b runs in bf16 with the length
    mask folded into the PSUM->SBUF copy (scalar_tensor_tensor).
Gate order is permuted host-side to [i, f, o, g]; per-step issue order is
f, i, g, o so the f-gate (first consumer) completes first.
"""
import sys
import numpy as np

sys.path.insert(0, "/opt/trn_rl_repo")

B, T, E, H, LH = 64, 512, 1024, 512, 512
S = 66
PN, WN, PD, WD = 32, 8, 64, 64
NCORES = 8
BL = B // NCORES
G4 = 4 * H  # 2048
KEMB = PN + WN + 1  # 41 (pos onehot, wordlen onehot, bias row)
NZP = 2  # zp prefetch depth

_PROGRAM_CACHE = {}


def _build_program():
    from concourse import bass, tile, mybir
    from concourse import bacc

    f32 = mybir.dt.float32
    bf16 = mybir.dt.bfloat16
    f8 = mybir.dt.float8e4
    AF = mybir.ActivationFunctionType
    ALU = mybir.AluOpType
    DR = mybir.MatmulPerfMode.DoubleRow
    f32r = mybir.dt.float32r
    R = lambda ap: ap.bitcast(f32r)

    nc = bacc.Bacc("TRN2", target_bir_lowering=False)

    # ---------------- I/O ----------------
    encT_b = nc.declare_dram_parameter("encT_b", [BL, E, T], f32r, isOutput=False)
    encF = nc.declare_dram_parameter("encF", [E, T * BL], bf16, isOutput=False)
    ATp = nc.declare_dram_parameter("ATp", [BL, T, T], f32r, isOutput=False)
    onehotT = nc.declare_dram_parameter("onehotT", [BL, KEMB, T], f32r, isOutput=False)
    Rm = nc.declare_dram_parameter("Rm", [KEMB, LH], f32r, isOutput=False)
    fcw1T = nc.declare_dram_parameter("fcw1T", [E, LH], f32r, isOutput=False)
    wihT = nc.declare_dram_parameter("wihT", [LH, G4], f32r, isOutput=False)
    whh8p = nc.declare_dram_parameter("whh8p", [128, 2, 2, G4], f8, isOutput=False)
    l9f8p = nc.declare_dram_parameter("l9f8p", [8, 2, 16], f8, isOutput=False)
    zpbias = nc.declare_dram_parameter("zpbias", [8, G4], f8, isOutput=False)
    ident8 = nc.declare_dram_parameter("ident8", [BL, BL], bf16, isOutput=False)
    combT = nc.declare_dram_parameter("combT", [H + E, S], bf16, isOutput=False)
    combB = nc.declare_dram_parameter("combB", [S, 1], f32, isOutput=False)
    mask66 = nc.declare_dram_parameter("mask66", [S, T * BL], f32, isOutput=False)
    outP = nc.declare_dram_parameter("out", [S, T * BL], f32, isOutput=True)

    # ---------------- internal HBM ----------------
    zp_hbm = nc.dram_tensor("zp_hbm", [T, BL, G4], f8)

    with tile.TileContext(nc) as tc:
        # ============ persistent pools (live across whole kernel) ============
        with (
            tc.tile_pool(name="pers", bufs=1) as pers,
            tc.tile_pool(name="ring", bufs=1) as ringp,
            tc.tile_pool(name="cell", bufs=2) as cell,
        ):
            # scan weights resident: whh fp8 DoubleRow layout [p, i, j, n]
            whh8 = pers.tile([128, 2, 2, G4], f8, tag="whh8", name="whh8")
            nc.sync.dma_start(whh8[:], whh8p[:])
            l9_sb = pers.tile([8, 2, 16], f8, tag="l9", name="l9s")
            nc.sync.dma_start(l9_sb[:], l9f8p[:])
            id8_sb = pers.tile([BL, BL], bf16, tag="id8", name="id8_sb")
            nc.sync.dma_start(id8_sb[:], ident8[:])
            # zp staging tiles: [:, 0, :] = per-step zp, [:, 1, :] = bias row
            zp9 = [pers.tile([8, 2, G4], f8, tag=f"zp9_{i}", name=f"zp9_{i}") for i in range(NZP)]
            for i in range(NZP):
                nc.sync.dma_start(zp9[i][:, 1, :], zpbias[:])
            # h^T rings, fp8: [128, slot(64), q(4 = j*2+i), b(16 padded)]
            ring = [ringp.tile([128, 64, 4, 16], f8, tag=f"ring{p}", name=f"ring{p}") for p in range(2)]
            nc.vector.memset(ring[0][:].bitcast(f32), 0.0)
            nc.vector.memset(ring[1][:].bitcast(f32), 0.0)
            # LSTM cell state (bf16)
            c_sb = pers.tile([BL, H], bf16, tag="c_sb", name="c_sb")
            nc.vector.memset(c_sb[:].bitcast(f32), 0.0)

            # ================= pre-phases (per-sequence) =================
            with (
                tc.tile_pool(name="pre_w", bufs=1) as pre_w,
                tc.tile_pool(name="pre_s", bufs=1) as pre_s,
                tc.tile_pool(name="ps_pre", bufs=4, space="PSUM") as ps_pre,
            ):
                fcw1_sb = [pre_w.tile([128, LH], f32r, tag=f"fcw1_{e}", name=f"fcw1_{e}") for e in range(8)]
                for e in range(8):
                    nc.sync.dma_start(fcw1_sb[e][:], fcw1T[e * 128:(e + 1) * 128, :])
                wih_sb = [pre_w.tile([128, G4], f32r, tag=f"wih{k}", name=f"wih{k}") for k in range(4)]
                for k in range(4):
                    nc.sync.dma_start(wih_sb[k][:], wihT[k * 128:(k + 1) * 128, :])
                rm_sb = pre_w.tile([KEMB, LH], f32r, tag="rm", name="rm")
                nc.sync.dma_start(rm_sb[:], Rm[:])

                for b in range(BL):
                    # ---- load enc^T for this sequence ----
                    enc_sb = [pre_s.tile([128, T], f32r, tag=f"enc{e}", name=f"enc{e}") for e in range(8)]
                    for e in range(8):
                        nc.sync.dma_start(enc_sb[e][:], encT_b[b, e * 128:(e + 1) * 128, :])
                    # ---- P = enc @ fc_w1^T  -> [T(s), LH] ----
                    P_sb = [pre_s.tile([128, LH], f32r, tag=f"P{sc}", name=f"P{sc}") for sc in range(4)]
                    for sc in range(4):
                        ps = ps_pre.tile([128, LH], f32, tag="psA", name="psA")
                        for e in range(8):
                            nc.tensor.matmul(
                                ps[:], R(enc_sb[e][:, sc * 128:(sc + 1) * 128]),
                                R(fcw1_sb[e][:]), start=(e == 0), stop=(e == 7),
                            )
                        nc.vector.tensor_copy(P_sb[sc][:], ps[:])
                    # ---- z^T = tanh(P^T A^T + R^T onehot^T) -> [LH, T] ----
                    at_sb = [pre_s.tile([128, T], f32r, tag=f"at{sc}", name=f"at{sc}") for sc in range(4)]
                    for sc in range(4):
                        nc.sync.dma_start(at_sb[sc][:], ATp[b, sc * 128:(sc + 1) * 128, :])
                    oh_sb = pre_s.tile([KEMB, T], f32r, tag="oh", name="oh")
                    nc.sync.dma_start(oh_sb[:], onehotT[b, :, :])
                    zT_sb = [pre_s.tile([128, T], f32r, tag=f"zT{m}", name=f"zT{m}") for m in range(4)]
                    for m in range(4):
                        ps = ps_pre.tile([128, T], f32, tag="psA", name="psA2")
                        for sc in range(4):
                            nc.tensor.matmul(
                                ps[:], R(P_sb[sc][:, m * 128:(m + 1) * 128]),
                                R(at_sb[sc][:]), start=(sc == 0), stop=False,
                            )
                        nc.tensor.matmul(
                            ps[:], R(rm_sb[:, m * 128:(m + 1) * 128]), R(oh_sb[:]),
                            start=False, stop=True,
                        )
                        nc.scalar.activation(zT_sb[m][:], ps[:], AF.Tanh)
                        nc.vector.memset(zT_sb[m][:, 0:1].bitcast(f32), 0.0)  # z_0 = 0
                    # ---- zp = z @ w_ih^T -> HBM [T, b, 4H] fp8 ----
                    for mt in range(4):
                        for nb in range(4):
                            ps = ps_pre.tile([128, 512], f32, tag="psB", name="psB")
                            for kc in range(4):
                                nc.tensor.matmul(
                                    ps[:], R(zT_sb[kc][:, mt * 128:(mt + 1) * 128]),
                                    R(wih_sb[kc][:, nb * 512:(nb + 1) * 512]),
                                    start=(kc == 0), stop=(kc == 3),
                                )
                            zpo = pre_s.tile([128, 512], f8, tag="zpo", name="zpo", bufs=2)
                            nc.vector.tensor_copy(zpo[:], ps[:])
                            nc.sync.dma_start(
                                zp_hbm[mt * 128:(mt + 1) * 128, b, nb * 512:(nb + 1) * 512],
                                zpo[:],
                            )

            tc.strict_bb_all_engine_barrier()

            # ================= LSTM scan (+ fused output projection) =================
            scan_ctx = [
                tc.tile_pool(name="ps_scan", bufs=1, space="PSUM"),
                tc.tile_pool(name="ps_tp", bufs=2, space="PSUM"),
                tc.tile_pool(name="fin_w", bufs=1),
                tc.tile_pool(name="fin_s", bufs=3),
            ]
            ps_scan = scan_ctx[0].__enter__()
            ps_tp = scan_ctx[1].__enter__()
            fin_w = scan_ctx[2].__enter__()
            fin_s = scan_ctx[3].__enter__()
            cw_sb = [fin_w.tile([128, S], bf16, tag=f"cw{k}", name=f"cw{k}") for k in range(12)]
            for k in range(12):
                nc.sync.dma_start(cw_sb[k][:], combT[k * 128:(k + 1) * 128, :])
            cb_sb = fin_w.tile([S, 1], f32, tag="cb", name="cb")
            nc.sync.dma_start(cb_sb[:], combB[:])
            mk_sb = fin_w.tile([S, T * BL], f32, tag="mk", name="mk")
            nc.sync.dma_start(mk_sb[:], mask66[:])
            for i in range(NZP):
                nc.sync.dma_start(zp9[i][:, 0, :], zp_hbm[i])

            # gate banks (host perm [f, i, g, o]): bank nb = cols [nb*512, ...).
            # All sigmoids are tanh half-angle: sigma(x) = 0.5*tanh(x/2) + 0.5,
            # with the 0.5 input scales folded into w_ih/w_hh/bias host-side,
            # and h' = 2h (folded into w_hh rows and comb_w rows).
            BANK_ORDER = [0, 1, 2, 3]  # f, i, g, o

            def inject(t, nb):
                pg = ps_scan.tile([128, 512], f32, tag="gates", name="pgb", bufs=5)
                nc.tensor.matmul(
                    pg[0:16, :], l9_sb[:],
                    zp9[t % NZP][:, :, nb * 512:(nb + 1) * 512],
                    start=True, stop=False, perf_mode=DR,
                )
                return pg

            psb = {nb: inject(0, nb) for nb in BANK_ORDER}
            for t in range(T):
                par = (t // 64) % 2
                slot = t % 64
                pprev = ((t - 1) // 64) % 2
                sprev = (t - 1) % 64

                # recurrent fp8 DoubleRow matmuls for step t (f first)
                for nb in BANK_ORDER:
                    for j in range(2):
                        nc.tensor.matmul(
                            psb[nb][0:16, :],
                            ring[pprev][:, sprev, 2 * j:2 * j + 2, :],
                            whh8[:, :, j, nb * 512:(nb + 1) * 512],
                            start=False, stop=(j == 1), perf_mode=DR,
                        )

                # PE p-state fillers: keep the array streaming while the cell
                # chain runs (PE drops to low clock on idle gaps otherwise).
                # Short 256-col fillers: similar duty to 5x512 but the worst-
                # case drain in front of the chain-critical c-transposes is
                # a quarter as long.
                dmy = ps_scan.tile([128, 512], f32, tag="gates", name="dmy", bufs=5)
                for _ in range(10):
                    nc.tensor.matmul(
                        dmy[0:16, 0:256],
                        ring[pprev][:, sprev, 0:2, :],
                        whh8[:, :, 0, 0:256],
                        start=True, stop=True, perf_mode=DR,
                    )

                # tanh-only cell, bf16:
                #   cs' = 2c;  m2t = (tf+1)*cs';  m1 = (ti+1)*tg
                #   cs'_new = 0.5*m2t + m1;  h' = (to^T+1)*tanh(0.5*cs')^T
                tf = cell.tile([BL, H], bf16, tag="tf", name="tf")
                ti = cell.tile([BL, H], bf16, tag="ti", name="ti")
                tg = cell.tile([BL, H], bf16, tag="tg", name="tg")
                to = cell.tile([BL, H], bf16, tag="to", name="to")
                m1 = cell.tile([BL, H], bf16, tag="m1", name="m1")
                m2t = cell.tile([BL, H], bf16, tag="m2t", name="m2t")
                tcT = cell.tile([128, 4, 8], bf16, tag="tcT", name="tcT")
                nc.scalar.activation(tf[:], psb[0][0:8, :], AF.Tanh)
                nc.scalar.activation(ti[:], psb[1][0:8, :], AF.Tanh)
                nc.scalar.activation(tg[:], psb[2][0:8, :], AF.Tanh)
                nc.scalar.activation(to[:], psb[3][0:8, :], AF.Tanh)
                nc.vector.scalar_tensor_tensor(
                    m2t[:], tf[:], 1.0, c_sb[:], op0=ALU.add, op1=ALU.mult)
                nc.vector.scalar_tensor_tensor(
                    m1[:], ti[:], 1.0, tg[:], op0=ALU.add, op1=ALU.mult)
                nc.vector.scalar_tensor_tensor(
                    c_sb[:], m2t[:], 0.5, m1[:], op0=ALU.mult, op1=ALU.add)
                psb_next = {}
                if t + 1 < T:
                    psb_next[0] = inject(t + 1, 0)
                    psb_next[1] = inject(t + 1, 1)

                # transposed tail
                tp = ps_tp.tile([128, 2, 4, 8], bf16, tag="tp", name="tp")
                for q in range(4):
                    nc.tensor.transpose(
                        tp[:, 0, q, :],
                        c_sb[:, q * 128:(q + 1) * 128], id8_sb[:],
                    )
                if t + 1 < T:
                    psb_next[2] = inject(t + 1, 2)
                for q in range(4):
                    nc.tensor.transpose(
                        tp[:, 1, q, :],
                        to[:, q * 128:(q + 1) * 128], id8_sb[:],
                    )
                if t + 1 < T:
                    psb_next[3] = inject(t + 1, 3)
                nc.scalar.activation(tcT[:], tp[:, 0, :, :], AF.Tanh, scale=0.5)
                nc.vector.scalar_tensor_tensor(
                    ring[par][:, slot, :, 0:8],
                    tp[:, 1, :, :], 1.0, tcT[:], op0=ALU.add, op1=ALU.mult)

                if slot == 63:
                    ncol = t // 64
                    cols = slice(ncol * 512, (ncol + 1) * 512)
                    rb = fin_s.tile([128, 4, 64, 8], bf16, tag="rb", name="rb")
                    for q in range(4):
                        nc.vector.tensor_copy(
                            rb[:, q, :, :], ring[par][:, :, q, 0:8],
                        )
                    psf = ps_tp.tile([S, 512], f32, tag="psF", name="psF", bufs=1)
                    for q in range(4):
                        nc.tensor.matmul(
                            psf[:], cw_sb[q][:], rb[:, q, :, :],
                            start=(q == 0), stop=False,
                        )
                    for ke in range(8):
                        re = fin_s.tile([128, 512], bf16, tag="re", name="re")
                        nc.sync.dma_start(re[:], encF[ke * 128:(ke + 1) * 128, cols])
                        nc.tensor.matmul(
                            psf[:], cw_sb[4 + ke][:], re[:], start=False, stop=(ke == 7),
                        )
                    ob = fin_s.tile([S, 512], f32, tag="ob", name="ob")
                    nc.vector.scalar_tensor_tensor(
                        ob[:], psf[:], cb_sb[:, 0:1], mk_sb[:, cols],
                        op0=ALU.add, op1=ALU.mult,
                    )
                    if ncol == 0:
                        nc.vector.memset(ob[0:1, 0:8], -1e30)
                    nc.sync.dma_start(outP[:, cols], ob[:])
                if t + NZP < T:
                    nc.sync.dma_start(zp9[t % NZP][:, 0, :], zp_hbm[t + NZP])
                psb = psb_next

            for c in reversed(scan_ctx):
                c.__exit__(None, None, None)

    nc.compile()
    return nc


def _host_prepack(inputs):
    """Build per-core in_maps from full inputs. Index plumbing + weight repacking only."""
    import ml_dtypes
    f8t = ml_dtypes.float8_e4m3
    b16t = ml_dtypes.bfloat16
    enc = np.asarray(inputs["encode_out"], np.float32)
    pos_emb_w = np.asarray(inputs["pos_emb_w"], np.float32)
    wl_emb_w = np.asarray(inputs["wordlen_emb_w"], np.float32)
    fc_w = np.asarray(inputs["fc_w"], np.float32)
    fc_b = np.asarray(inputs["fc_b"], np.float32)
    w_ih = np.asarray(inputs["w_ih"], np.float32)
    w_hh = np.asarray(inputs["w_hh"], np.float32)
    b_ih = np.asarray(inputs["b_ih"], np.float32)
    b_hh = np.asarray(inputs["b_hh"], np.float32)
    comb_w = np.asarray(inputs["comb_w"], np.float32)
    comb_b = np.asarray(inputs["comb_b"], np.float32)
    pos_ids = np.asarray(inputs["pos_ids"])
    wl_ids = np.asarray(inputs["wordlen_ids"])
    wl_vals = np.asarray(inputs["wordlen_vals"])
    lengths = np.asarray(inputs["lengths"])

    t = np.arange(T)
    # gate permutation [f, i, g, o]; tanh half-angle scales: f,i,o inputs x0.5
    perm = np.r_[H:2 * H, 0:H, 2 * H:3 * H, 3 * H:4 * H]
    colscale = np.concatenate(
        [np.full(H, 0.5), np.full(H, 0.5), np.ones(H), np.full(H, 0.5)]
    ).astype(np.float32)
    wihT = np.ascontiguousarray(w_ih[perm].T) * colscale[None, :]  # [LH, 4H]
    # w_hh rows x0.5 compensates h' = 2h
    whhT = np.ascontiguousarray(w_hh[perm].T) * colscale[None, :] * 0.5
    # fp8 DoubleRow layout [p, i, j, n]: row K = j*256 + i*128 + p
    whh8 = np.ascontiguousarray(
        whhT.reshape(2, 2, 128, G4).transpose(2, 1, 0, 3)).astype(f8t)
    # inject stationary: [p, 0, m] = identity; [0, 1, m] = 1 (bias row)
    l9f8 = np.zeros((8, 2, 16), np.float32)
    l9f8[:, 0, :8] = np.eye(8)
    l9f8[0, 1, :8] = 1.0
    l9f8 = l9f8.astype(f8t)
    # bias row for zp9[:, 1, :]: partition 0 = b_ih + b_hh, rest 0
    zpbias = np.zeros((8, G4), np.float32)
    zpbias[0] = (b_ih + b_hh)[perm] * colscale
    zpbias = zpbias.astype(f8t)
    # embedding folding
    fc_w1 = fc_w[:, :E]
    fc_w2 = fc_w[:, E:E + WD]
    fc_w3 = fc_w[:, E + WD:]
    Rm = np.concatenate(
        [pos_emb_w @ fc_w3.T, wl_emb_w @ fc_w2.T, fc_b[None, :]], axis=0
    ).astype(np.float32)  # [41, LH]
    fcw1T = np.ascontiguousarray(fc_w1.T)  # [E, LH]
    combT = np.ascontiguousarray(comb_w.T)  # [1536, S]
    combT[:H] *= 0.5  # h' = 2h compensation
    combT = combT.astype(b16t)
    combB = comb_b[:, None].astype(np.float32)

    in_maps = []
    for c in range(NCORES):
        bs = slice(c * BL, (c + 1) * BL)
        enc_sh = enc[bs]  # [BL, T, E]
        encT_b = np.ascontiguousarray(enc_sh.transpose(0, 2, 1))  # [BL, E, T]
        encF = np.ascontiguousarray(
            enc_sh.transpose(2, 1, 0).reshape(E, T * BL)).astype(b16t)
        # selection matrix A^T per sequence
        ATp = np.zeros((BL, T, T), np.float32)
        oneh = np.zeros((BL, KEMB, T), np.float32)
        for j in range(BL):
            wv = wl_vals[bs][j].astype(np.int64)
            start = np.clip(t - wv, 0, None)
            denom = np.maximum(t - start, 1).astype(np.float32)
            s = np.arange(T)[:, None]
            m = (s >= start[None, :]) & (s < t[None, :])
            ATp[j] = m / denom[None, :]
            oneh[j][pos_ids[bs][j].astype(np.int64), t] = 1.0
            oneh[j][PN + wl_ids[bs][j].astype(np.int64), t] = 1.0
            oneh[j][KEMB - 1, :] = 1.0
        maskv = (t[:, None] < lengths[bs][None, :]).astype(np.float32)  # [T, BL]
        mask66 = np.broadcast_to(maskv.reshape(1, T * BL), (S, T * BL)).copy()
        in_maps.append({
            "encT_b": encT_b, "encF": encF, "ATp": ATp, "onehotT": oneh,
            "Rm": Rm, "fcw1T": fcw1T, "wihT": wihT, "whh8p": whh8,
            "l9f8p": l9f8, "zpbias": zpbias,
            "ident8": np.eye(BL, dtype=b16t),
            "combT": combT, "combB": combB,
            "mask66": mask66,
        })
    return in_maps


def kernel(**inputs):
    from concourse.bass_utils import run_bass_kernel_spmd

    if "prog" not in _PROGRAM_CACHE:
        _PROGRAM_CACHE["prog"] = _build_program()
    nc = _PROGRAM_CACHE["prog"]
    in_maps = _host_prepack(inputs)
    res = run_bass_kernel_spmd(nc, in_maps, list(range(NCORES)))
    outs = []
    for c in range(NCORES):
        o = np.asarray(res.results[c]["out"])  # [S, T*BL]
        outs.append(o.reshape(S, T, BL).transpose(2, 1, 0))  # [BL, T, S]
    return np.ascontiguousarray(np.concatenate(outs, axis=0)).astype(np.float32)
